# revision 1
# baseline (speedup 1.0000x reference)
"""CTC loss kernel for Trainium2 (Bass/Tile), 8-core data parallel.

Computes the reference's log-space CTC forward DP exactly:

    A_t[s] = lp_t[s] + logaddexp(logaddexp(A[s], A[s-1]), A[s-2] + mneg[s])

with lp = ln(y_pred + EPS), mneg[s] = 0 where the skip transition is
allowed and -1e30 (log-zero) where it is not.  Each logaddexp(x, y) =
max(x,y) + softplus(min(x,y) - max(x,y)), softplus = Ln(Exp(d) + 1) on
the ScalarEngine (both funcs live in one activation table).

Layout: the 129 states are packed as 4 chunks x 32 batches across the
128 partitions.  Each partition row holds [2 NEG pads | 16 overlap
states | 33 real states] = 51 columns, so every vector-engine op is 49
columns instead of 129.  The s-1/s-2 shifts stay in-lane; the overlap
region is recomputed redundantly (it equals the previous chunk's top
states) and drifts from truth by at most 2 states/step starting at the
NEG pads, so a single partition-shifted SBUF->SBUF DMA every 8 steps
(copying the upstream chunk's top 16 real states into the downstream
overlap) keeps all real states exact.  Virtual/out-of-range states sit
at -1e30, which float-absorbs all updates (as in the reference).

The per-symbol log-probs lp[b,t,s] = ln(y_pred[b,t,ext[b,s]] + EPS) are
gathered on-device with TensorEngine matmuls against one-hot matrices
G[c, (b,s)] = (c == ext[b,s]) (zero columns for virtual states), the Ln
fused into the PSUM->SBUF copy, staged through a DRAM scratch, and
streamed back in the packed layout.  G, the skip mask and the transpose
identity are tiny index-preprocessing artifacts of y_true prepared
host-side.
"""

import numpy as np

import concourse.bass as bass
import concourse.tile as tile
from concourse import bacc
from concourse import mybir
from concourse.bass_utils import run_bass_kernel_spmd
from contextlib import ExitStack

B, T, C, L = 256, 1024, 128, 64
NCORES = 8
BPC = B // NCORES          # 32 batch rows per core
S = 2 * L + 1              # 129 extended states
NCH, CSZ = 4, 33           # state chunks per batch
W = 16                     # overlap states per chunk
SEXT = W + NCH * CSZ       # 148: left-padded + padded state axis in LP
NST = W + CSZ              # 49 computed states per row
RFR = 8                    # overlap refresh period (2 states/step drift)
BLANK = C - 1              # 127
EPS = 1e-7
NEG = -1.0e30              # log-space zero (matches reference)
TC = 64                    # DP stream chunk: time steps per SBUF tile
OCT = 128                  # pregather granularity: time steps per matmul

f32 = mybir.dt.float32
Alu = mybir.AluOpType
Act = mybir.ActivationFunctionType

# This kernel only uses Exp / Ln / Copy / Identity activations, all present in
# the single "natural_log_exp_and_others" table.  The act-table placement pass
# greedily picks the first table containing each func (exp_and_others for Exp,
# natural_log for Ln), thrashing a 1.3us table load around every Exp<->Ln
# transition (~4096 loads).  Blank every other table (ids/positions preserved)
# so the pass settles on the combined table once.
_orig_get_act_tables = bacc.get_activation_tables


def _patched_get_act_tables(arch):
    tabs = _orig_get_act_tables(arch)
    keep = "natural_log_exp_and_others"
    if keep in tabs:
        tabs = {n: (fs if n == keep else set()) for n, fs in tabs.items()}
    return tabs


bacc.get_activation_tables = _patched_get_act_tables


def _build() -> bass.Bass:
    nc = bacc.Bacc()
    y_pred = nc.dram_tensor("y_pred", [BPC, T, C], f32, kind="ExternalInput")
    g_in = nc.dram_tensor("g_all", [C, BPC * SEXT], f32, kind="ExternalInput")
    m2_in = nc.dram_tensor("m2neg", [128, NST], f32, kind="ExternalInput")
    id_in = nc.dram_tensor("ident", [128, 128], f32, kind="ExternalInput")
    eps_in = nc.dram_tensor("eps_bias", [128, 1], f32, kind="ExternalInput")
    loss = nc.dram_tensor("loss", [BPC, 1], f32, kind="ExternalOutput")

    with tile.TileContext(nc) as tc, ExitStack() as ctx:
        persist = ctx.enter_context(tc.tile_pool(name="persist", bufs=1))
        tmp = ctx.enter_context(tc.tile_pool(name="tmp", bufs=3))
        ysb = ctx.enter_context(tc.tile_pool(name="ysb", bufs=3))
        ytp = ctx.enter_context(tc.tile_pool(name="ytp", bufs=3))
        pstream = ctx.enter_context(tc.tile_pool(name="pstream", bufs=3))
        psum_tp = ctx.enter_context(tc.tile_pool(name="psum_tp", bufs=2, space="PSUM"))
        psum_pp = ctx.enter_context(tc.tile_pool(name="psum_pp", bufs=2, space="PSUM"))
        psum_d = ctx.enter_context(tc.tile_pool(name="psum_d", bufs=1, space="PSUM"))
        psum_r = ctx.enter_context(tc.tile_pool(name="psum_r", bufs=2, space="PSUM"))
        dram = ctx.enter_context(tc.tile_pool(name="dram", bufs=1, space="DRAM"))

        # ---------- load static inputs ----------
        ident = persist.tile([128, 128], f32, tag="ident")
        nc.sync.dma_start(out=ident, in_=id_in[:, :])
        g_all = persist.tile([C, BPC * SEXT], f32, tag="gall")
        nc.sync.dma_start(out=g_all, in_=g_in[:, :])
        m2neg = persist.tile([128, NST], f32, tag="m2")
        nc.sync.dma_start(out=m2neg, in_=m2_in[:, :])
        eps_bias = persist.tile([128, 1], f32, tag="epsb")
        nc.sync.dma_start(out=eps_bias, in_=eps_in[:, :])

        # Dummy PE ops: absorb the ident / g_all DMA waits so that every
        # later PE instruction carries at most one sync wait.
        d1ps = psum_d.tile([128, 128], f32, tag="d1")
        nc.tensor.transpose(d1ps, ident, ident)
        d2ps = psum_d.tile([128, 1], f32, tag="d2")
        nc.tensor.matmul(d2ps, lhsT=g_all[:, 0:128], rhs=g_all[:, 0:1],
                         start=True, stop=True)

        # ---------- pregather: LP[b,t,sx] = ln(y_pred[b,t,ext[b,sx]] + EPS) --
        # sx axis: 16 virtual left states + 129 real + 3 dead (all-zero G
        # columns -> lp = ln(EPS) there; they never affect real states).
        p_oct = [
            dram.tile([BPC, OCT, SEXT], f32, tag=f"oct{o}", name=f"p_oct{o}")
            for o in range(T // OCT)
        ]
        for o in range(T // OCT):
            for b in range(BPC):
                y_sb = ysb.tile([OCT, C], f32, tag="y")
                nc.sync.dma_start(out=y_sb, in_=y_pred[b, o * OCT : (o + 1) * OCT, :])
                yT_ps = psum_tp.tile([C, OCT], f32, tag="tp")
                nc.tensor.transpose(yT_ps, y_sb, ident)
                yT_sb = ytp.tile([C, OCT], f32, tag="yT")
                nc.scalar.activation(out=yT_sb, in_=yT_ps, func=Act.Copy)
                p_ps = psum_pp.tile([OCT, SEXT], f32, tag="pp")
                nc.tensor.matmul(
                    p_ps, lhsT=yT_sb, rhs=g_all[:, b * SEXT : (b + 1) * SEXT],
                    start=True, stop=True,
                )
                p_sb = ytp.tile([OCT, SEXT], f32, tag="psb")
                nc.scalar.activation(
                    out=p_sb, in_=p_ps, func=Act.Ln, bias=eps_bias[:, :]
                )
                nc.sync.dma_start(out=p_oct[o][b, :, :], in_=p_sb)

        # ---------- DP over time (log space, packed 4x32 partitions) -------
        # row p = 32*k + b: chunk k of batch b; cols 0,1 NEG pads; cols
        # 2..17 overlap (states 33k-16..33k-1); cols 18..50 real states
        # 33k..33k+32.
        a_pads = [
            persist.tile([128, NST + 2], f32, tag=f"alpha{i}", name=f"alpha{i}")
            for i in range(2)
        ]
        q_pads = [
            persist.tile([128, NST + 2], f32, tag=f"qres{i}", name=f"qres{i}")
            for i in range(2)
        ]
        nc.vector.memset(a_pads[0], NEG)
        nc.vector.memset(a_pads[1], NEG)
        nc.vector.memset(q_pads[0], 0.0)
        nc.vector.memset(q_pads[1], 0.0)

        FOLD = 32              # q in [1, 3^FOLD] < f32 max; fold bubbles rare

        def shift_refresh(tile_, name):
            # overlap re-sync via PE partition shift: out[32+p] = in[p] using
            # a 96x96 identity as weights and a partition-offset output AP,
            # then ACT copies PSUM back to SBUF lane-aligned.  ~3x lower
            # latency than an SBUF->SBUF DMA (no 900ns DMA sem propagation).
            nc.sync.dma_start(
                out=tile_[32:128, 2 : 2 + W],
                in_=tile_[0:96, 2 + NST - W : 2 + NST],
            )
        def a_seg(t, lp):
            # m-side: mxx = max3(m0, m1, s2m); d_i = term_i - mxx;
            # m' = mxx + lp.  Depends only on the m tiles.
            src = a_pads[(t + 1) % 2]
            dst = a_pads[t % 2]
            m0 = src[:, 2 : 2 + NST]
            m1 = src[:, 1 : 1 + NST]
            m2v = src[:, 0:NST]
            s2m = tmp.tile([128, NST], f32, tag="s2m", name=f"s2m{t%4}")
            nc.gpsimd.tensor_add(out=s2m, in0=m2v, in1=m2neg)
            mxa = tmp.tile([128, NST], f32, tag="mxa", name=f"mxa{t%4}")
            nc.vector.tensor_max(out=mxa, in0=m0, in1=m1)
            mxx = tmp.tile([128, NST], f32, tag="mxx", name=f"mxx{t%4}")
            nc.vector.tensor_max(out=mxx, in0=mxa, in1=s2m)
            # m' first: the next step's a_seg depends only on this write, so
            # the m-recurrence critical path is 4 ops; d's fill the slack
            nc.vector.tensor_add(out=dst[:, 2 : 2 + NST], in0=mxx, in1=lp)
            d0 = tmp.tile([128, NST], f32, tag="d0", name=f"d0_{t%4}")
            nc.vector.tensor_sub(out=d0, in0=m0, in1=mxx)
            d1 = tmp.tile([128, NST], f32, tag="d1", name=f"d1_{t%4}")
            nc.gpsimd.tensor_sub(out=d1, in0=m1, in1=mxx)
            d2 = tmp.tile([128, NST], f32, tag="d2", name=f"d2_{t%4}")
            nc.gpsimd.tensor_sub(out=d2, in0=s2m, in1=mxx)
            return (t, d0, d1, d2)

        def x_seg(st):
            t, d0, d1, d2 = st
            x0 = tmp.tile([128, NST], f32, tag="x0", name=f"x0_{t%4}")
            nc.scalar.activation(out=x0, in_=d0, func=Act.Exp)
            x1 = tmp.tile([128, NST], f32, tag="x1", name=f"x1_{t%4}")
            nc.scalar.activation(out=x1, in_=d1, func=Act.Exp)
            x2 = tmp.tile([128, NST], f32, tag="x2", name=f"x2_{t%4}")
            nc.scalar.activation(out=x2, in_=d2, func=Act.Exp)
            return (t, x0, x1, x2)

        def q_seg(st):
            t, x0, x1, x2 = st
            srcq = q_pads[(t + 1) % 2]
            dstq = q_pads[t % 2]
            q0 = srcq[:, 2 : 2 + NST]
            q1 = srcq[:, 1 : 1 + NST]
            q2 = srcq[:, 0:NST]
            t0 = tmp.tile([128, NST], f32, tag="t0", name=f"t0_{t%4}")
            nc.vector.tensor_mul(out=t0, in0=q0, in1=x0)
            t1 = tmp.tile([128, NST], f32, tag="t1", name=f"t1_{t%4}")
            nc.vector.tensor_mul(out=t1, in0=q1, in1=x1)
            t01 = tmp.tile([128, NST], f32, tag="t01", name=f"t01_{t%4}")
            nc.vector.tensor_add(out=t01, in0=t0, in1=t1)
            t2 = tmp.tile([128, NST], f32, tag="t2", name=f"t2_{t%4}")
            nc.gpsimd.tensor_mul(out=t2, in0=q2, in1=x2)
            nc.gpsimd.tensor_add(out=dstq[:, 2 : 2 + NST], in0=t01, in1=t2)

        def fold_and_refresh(t):
            # fold q into m on the real columns, reset q, then re-sync the
            # m overlap from the folded reals and set the q overlap to 1.
            dst = a_pads[t % 2]
            dstq = q_pads[t % 2]
            rc = slice(2 + W, 2 + NST)
            qc = tmp.tile([128, CSZ], f32, tag="qc")
            nc.vector.tensor_scalar_max(qc, dstq[:, rc], 1e-30)
            lq = tmp.tile([128, CSZ], f32, tag="lq")
            nc.scalar.activation(out=lq, in_=qc, func=Act.Ln)
            nc.vector.tensor_add(out=dst[:, rc], in0=dst[:, rc], in1=lq)
            nc.vector.memset(dstq[:, rc], 1.0)
            shift_refresh(dst, f"rfm{t%2}")
            nc.vector.memset(dstq[:, 2 : 2 + W], 1.0)

        pending = None       # x-stage state of step t-1 awaiting its q_seg
        for c in range(T // TC):
            pt = pstream.tile([128, TC, NST], f32, tag="ps", name=f"pt{c%4}")
            o, h = divmod(c, OCT // TC)
            for k in range(NCH):
                nc.sync.dma_start(
                    out=pt[32 * k : 32 * (k + 1), :, :],
                    in_=p_oct[o][:, h * TC : (h + 1) * TC, 33 * k : 33 * k + NST],
                )
            if c == 0:
                # alpha_0: only s=0 (blank) and s=1 (first label) reachable
                nc.vector.tensor_copy(
                    out=a_pads[0][0:32, W + 2 : W + 4], in_=pt[0:32, 0, W : W + 2]
                )
                nc.vector.memset(q_pads[0][0:32, W + 2 : W + 4], 1.0)
            for tl in range(1 if c == 0 else 0, TC):
                t = c * TC + tl
                lp = pt[:, tl, :]
                tp = t - 1
                if pending is not None and tp % FOLD == 0:
                    # pipeline drain: step t must consume the folded m(t-1)
                    q_seg(pending)
                    pending = None
                    fold_and_refresh(tp)
                st = a_seg(t, lp)
                if t % RFR == 0 and t % FOLD != 0:
                    # re-sync m overlaps (q overlaps synced after q_seg(t))
                    shift_refresh(a_pads[t % 2], f"rm{t%2}")
                xs = x_seg(st)
                if pending is not None:
                    q_seg(pending)
                    if tp % RFR == 0 and tp % FOLD != 0:
                        shift_refresh(q_pads[tp % 2], f"rq{tp%2}")
                pending = xs
        q_seg(pending)

        # ---------- epilogue: loss = -logaddexp(A[127], A[128]) ----------
        # states 127,128 = chunk 3 reals 28,29 -> partitions 96..127,
        # cols 2+W+28=46, 47.  A = m + ln(q).
        a_fin = a_pads[(T - 1) % 2]
        q_fin = q_pads[(T - 1) % 2]
        qcf = persist.tile([128, 2], f32, tag="qcf")
        nc.vector.tensor_scalar_max(qcf[96:128, :], q_fin[96:128, 46:48], 1e-30)
        lqf = persist.tile([128, 2], f32, tag="lqf")
        nc.scalar.activation(out=lqf[96:128, :], in_=qcf[96:128, :], func=Act.Ln)
        af = persist.tile([128, 2], f32, tag="af")
        nc.vector.tensor_add(out=af[96:128, :], in0=a_fin[96:128, 46:48], in1=lqf[96:128, :])
        e0 = af[96:128, 0:1]
        e1 = af[96:128, 1:2]
        mxf = persist.tile([128, 1], f32, tag="mxf")
        nc.vector.tensor_max(out=mxf[96:128, :], in0=e0, in1=e1)
        mnf = persist.tile([128, 1], f32, tag="mnf")
        nc.vector.tensor_tensor(out=mnf[96:128, :], in0=e0, in1=e1, op=Alu.min)
        ddf = persist.tile([128, 1], f32, tag="ddf")
        nc.vector.tensor_sub(out=ddf[96:128, :], in0=mnf[96:128, :], in1=mxf[96:128, :])
        exf = persist.tile([128, 1], f32, tag="exf")
        nc.scalar.activation(out=exf[96:128, :], in_=ddf[96:128, :], func=Act.Exp)
        spf = persist.tile([128, 1], f32, tag="spf")
        nc.scalar.activation(out=spf[96:128, :], in_=exf[96:128, :], func=Act.Ln, bias=1.0)
        out_t = persist.tile([128, 1], f32, tag="outt")
        # loss = -(mxf + spf)
        nc.vector.scalar_tensor_tensor(
            out=out_t[96:128, :], in0=mxf[96:128, :], scalar=-1.0, in1=spf[96:128, :],
            op0=Alu.mult, op1=Alu.subtract,
        )
        nc.sync.dma_start(out=loss[:, :], in_=out_t[96:128, :])

    nc.finalize()
    return nc


def _host_prep(y_true: np.ndarray):
    """Tiny index-preprocessing of y_true: one-hot gather matrices (axis =
    16 virtual + 129 real + 3 dead states) and the packed skip mask."""
    ext = np.full((B, S), BLANK, np.int32)
    ext[:, 1::2] = y_true
    g = np.zeros((B, C, SEXT), np.float32)
    g[:, :, W : W + S] = ext[:, None, :] == np.arange(C, dtype=np.int32)[None, :, None]
    m2 = np.zeros((B, S), np.bool_)
    m2[:, 3::2] = y_true[:, 1:] != y_true[:, :-1]
    mfull = np.full((B, SEXT), np.float32(NEG), np.float32)
    mfull[:, W : W + S] = np.where(m2, np.float32(0.0), np.float32(NEG))
    # packed per-row mask: partition p = 32k+b covers states 33k-16..33k+32
    # = mfull cols 33k..33k+48
    mrows = np.stack(
        [mfull[:, 33 * k : 33 * k + NST] for k in range(NCH)], axis=0
    ).reshape(128, NST)
    return g, mrows


_NC = None
LAST_RESULT = None


def kernel(y_true: np.ndarray, y_pred: np.ndarray) -> np.ndarray:
    global _NC, LAST_RESULT
    if _NC is None:
        _NC = _build()
    y_true = np.asarray(y_true, dtype=np.int32)
    y_pred = np.ascontiguousarray(np.asarray(y_pred, dtype=np.float32))
    ident = np.eye(128, dtype=np.float32)
    eps_bias = np.full((128, 1), EPS, np.float32)
    in_maps = []
    for i in range(NCORES):
        sl = slice(i * BPC, (i + 1) * BPC)
        g, mrows = _host_prep_core(y_true[sl])
        in_maps.append(
            {
                "y_pred": y_pred[sl],
                "g_all": g,
                "m2neg": mrows,
                "ident": ident,
                "eps_bias": eps_bias,
            }
        )
    res = run_bass_kernel_spmd(_NC, in_maps, core_ids=list(range(NCORES)))
    LAST_RESULT = res
    return np.concatenate([r["loss"] for r in res.results], axis=0)


def _host_prep_core(y_true_c: np.ndarray):
    ext = np.full((BPC, S), BLANK, np.int32)
    ext[:, 1::2] = y_true_c
    g = np.zeros((BPC, C, SEXT), np.float32)
    g[:, :, W : W + S] = (
        ext[:, None, :] == np.arange(C, dtype=np.int32)[None, :, None]
    )
    g = np.ascontiguousarray(g.transpose(1, 0, 2).reshape(C, BPC * SEXT))
    m2 = np.zeros((BPC, S), np.bool_)
    m2[:, 3::2] = y_true_c[:, 1:] != y_true_c[:, :-1]
    mfull = np.full((BPC, SEXT), np.float32(NEG), np.float32)
    mfull[:, W : W + S] = np.where(m2, np.float32(0.0), np.float32(NEG))
    mrows = np.ascontiguousarray(
        np.stack([mfull[:, 33 * k : 33 * k + NST] for k in range(NCH)], axis=0)
        .reshape(128, NST)
    )
    return g, mrows



# revision 7
# speedup vs baseline: 1.5346x; 1.5346x over previous
"""CTC loss kernel for Trainium2 (Bass/Tile), 8-core data parallel.

Linear-space CTC forward DP with periodic per-row renormalization:

    a_t[s] = (a[s] + a[s-1] + m2[s]*a[s-2]) * ptil_t[s]

where ptil = (y_pred + EPS) * KP, KP ~ e^{E[-dloss/dt]} chosen so the row
magnitude is drift-free on average.  Every R=8 steps each partition row is
rescaled by KC/max(rowsum, 1) (rowsum from the stt accumulator two steps
earlier, reciprocal on DVE), and the applied log-scales accumulate into a
per-row f32 accumulator; the final loss is -(ln(a[127]+a[128]) + acc -
T*ln(KP)).  All hot-loop tensors are bf16 (2x DVE mode); states that fall
~90 nats below their row maximum flush to zero, which the 2e-2 rel-err
budget tolerates by a wide margin (validated: max rel err ~2e-4).

Layout: 129 states packed as 4 chunks x 32 batches across 128 partitions.
Each row holds [2 zero pads | 16 overlap | 33 real] = 51 bf16 cols, so the
s-1/s-2 shifts stay in-lane.  The overlap is recomputed redundantly and
drifts 2 cols/step from the pads; every 8 steps a PE shift-matmul copies the
upstream chunk's top-16 states into the downstream overlap, rescaled by
exp(acc_upstream - acc_this) via an ACT per-partition-scaled copy.  Rows
whose states are still all-zero (unreachable chunks) instead adopt the
upstream accumulator so arriving values always land in f32 range.

The per-symbol probs ptil[b,t,s] are gathered on-device by TensorEngine
matmuls against one-hot matrices G[c,(b,sx)] = KP*(c==ext[b,sx]) in bf16,
staged through a DRAM scratch, and streamed back in the packed layout.
"""

import numpy as np

import concourse.bass as bass
import concourse.tile as tile
from concourse import bacc
from concourse import mybir
from concourse.bass_utils import run_bass_kernel_spmd
from contextlib import ExitStack

B, T, C, L = 256, 1024, 128, 64
NCORES = 8
BPC = B // NCORES          # 32 batch rows per core
S = 2 * L + 1              # 129 extended states
NCH = 4                    # state chunks per batch
W = 16                     # overlap states per chunk
N = W + 33                 # 49 computed states per row
SEXT = W + S + 3           # 148: padded per-batch state axis in the gather
R = 8                      # renorm + refresh period
BLANK = C - 1
EPS = 1e-7
KP = 108.0                 # folded into G: ptil = (y+EPS)*KP, E[step drift]~0
KC = float(2.0 ** 17)      # renorm target row sum
TC = 64                    # DP stream chunk: time steps per SBUF tile
OCT = 128                  # pregather granularity: time steps per matmul

f32 = mybir.dt.float32
bf16 = mybir.dt.bfloat16
Alu = mybir.AluOpType
Act = mybir.ActivationFunctionType

# Only Copy / Exp / Ln are used, all present in the single
# "natural_log_exp_and_others" table.  Blank every other table so the
# act-table placement pass settles on the combined table once (avoids a
# 1.3us table load on every Exp<->Ln transition).
_orig_get_act_tables = bacc.get_activation_tables


def _patched_get_act_tables(arch):
    tabs = _orig_get_act_tables(arch)
    keep = "natural_log_exp_and_others"
    if keep in tabs:
        tabs = {n: (fs if n == keep else set()) for n, fs in tabs.items()}
    return tabs


bacc.get_activation_tables = _patched_get_act_tables


def _build() -> bass.Bass:
    nc = bacc.Bacc()
    y_pred = nc.dram_tensor("y_pred", [BPC, T, C], f32, kind="ExternalInput")
    g_in = nc.dram_tensor("g_all", [C, BPC * SEXT], bf16, kind="ExternalInput")
    m2_in = nc.dram_tensor("m2mask", [128, N], bf16, kind="ExternalInput")
    id_in = nc.dram_tensor("ident", [128, 128], f32, kind="ExternalInput")
    shst_in = nc.dram_tensor("shst", [128, 128], bf16, kind="ExternalInput")
    shacc_in = nc.dram_tensor("shacc", [128, 128], f32, kind="ExternalInput")
    loss = nc.dram_tensor("loss", [BPC, 1], f32, kind="ExternalOutput")

    with tile.TileContext(nc) as tc, ExitStack() as ctx:
        persist = ctx.enter_context(tc.tile_pool(name="persist", bufs=1))
        tmp = ctx.enter_context(tc.tile_pool(name="tmp", bufs=3))
        ysb = ctx.enter_context(tc.tile_pool(name="ysb", bufs=3))
        ytp = ctx.enter_context(tc.tile_pool(name="ytp", bufs=3))
        pstream = ctx.enter_context(tc.tile_pool(name="pstream", bufs=3))
        psum_tp = ctx.enter_context(tc.tile_pool(name="psum_tp", bufs=2, space="PSUM"))
        psum_pp = ctx.enter_context(tc.tile_pool(name="psum_pp", bufs=2, space="PSUM"))
        psum_r = ctx.enter_context(tc.tile_pool(name="psum_r", bufs=2, space="PSUM"))
        dram = ctx.enter_context(tc.tile_pool(name="dram", bufs=1, space="DRAM"))

        # ---------- static inputs ----------
        ident = persist.tile([128, 128], f32, tag="ident")
        nc.sync.dma_start(out=ident, in_=id_in[:, :])
        g_all = persist.tile([C, BPC * SEXT], bf16, tag="gall")
        nc.sync.dma_start(out=g_all, in_=g_in[:, :])
        m2 = persist.tile([128, N], bf16, tag="m2")
        nc.sync.dma_start(out=m2, in_=m2_in[:, :])
        shst = persist.tile([128, 128], bf16, tag="shst")
        nc.sync.dma_start(out=shst, in_=shst_in[:, :])
        shacc = persist.tile([128, 128], f32, tag="shacc")
        nc.sync.dma_start(out=shacc, in_=shacc_in[:, :])


        # ---------- pregather: ptil[b,t,sx] = (y_pred[b,t,ext[b,sx]]+EPS)*KP
        p_oct = [
            dram.tile([BPC, OCT, SEXT], bf16, tag=f"oct{o}", name=f"p_oct{o}")
            for o in range(T // OCT)
        ]
        for o in range(T // OCT):
            for b in range(BPC):
                y_sb = ysb.tile([OCT, C], f32, tag="y")
                nc.sync.dma_start(out=y_sb, in_=y_pred[b, o * OCT : (o + 1) * OCT, :])
                yT_ps = psum_tp.tile([C, OCT], f32, tag="tp")
                nc.tensor.transpose(yT_ps, y_sb, ident)
                yT_sb = ytp.tile([C, OCT], bf16, tag="yT")
                nc.scalar.activation(out=yT_sb, in_=yT_ps, func=Act.Copy)
                p_ps = psum_pp.tile([OCT, SEXT], f32, tag="pp")
                nc.tensor.matmul(
                    p_ps, lhsT=yT_sb, rhs=g_all[:, b * SEXT : (b + 1) * SEXT],
                    start=True, stop=True,
                )
                p_sb = ytp.tile([OCT, SEXT], bf16, tag="psb")
                nc.scalar.activation(
                    out=p_sb, in_=p_ps, func=Act.Copy, bias=float(KP * EPS)
                )
                nc.scalar.dma_start(out=p_oct[o][b, :, :], in_=p_sb)

        # ---------- DP state ----------
        a_pads = [
            persist.tile([128, N + 2], bf16, tag=f"alpha{i}", name=f"alpha{i}")
            for i in range(2)
        ]
        nc.vector.memset(a_pads[0], 0.0)
        nc.vector.memset(a_pads[1], 0.0)
        acc = persist.tile([128, 1], f32, tag="acc")
        nc.vector.memset(acc, 0.0)
        nrs = 2 * (T // (2 * R)) + 2
        rsum_t = [persist.tile([128, 1], f32, tag=f"rs{i%4}", name=f"rs{i}") for i in range(nrs)]
        rg_t = [persist.tile([128, 1], f32, tag=f"rg{i%4}", name=f"rg{i}") for i in range(nrs)]
        rinv_t = [persist.tile([128, 1], f32, tag=f"ri{i%4}", name=f"ri{i}") for i in range(nrs)]
        isd_t = [persist.tile([128, 1], f32, tag=f"is{i%4}", name=f"is{i}") for i in range(nrs)]
        lnrg_t = [persist.tile([128, 1], f32, tag=f"ln{i%4}", name=f"ln{i}") for i in range(nrs)]

        def step(t, lp):
            src = a_pads[(t + 1) % 2]
            dst = a_pads[t % 2]
            a0 = src[:, 2 : 2 + N]
            a1 = src[:, 1 : 1 + N]
            a2 = src[:, 0:N]
            u = tmp.tile([128, N], bf16, tag="u", name=f"u{t%4}")
            nc.vector.tensor_add(out=u, in0=a0, in1=a1)
            t2 = tmp.tile([128, N], bf16, tag="t2", name=f"t2_{t%4}")
            nc.vector.tensor_mul(out=t2, in0=a2, in1=m2)
            w = tmp.tile([128, N], bf16, tag="w", name=f"w{t%4}")
            nc.vector.tensor_add(out=w, in0=u, in1=t2)
            j = t // R
            if t % R == R - 2 and t + 2 < T:
                # emit row sum for the correction two steps later
                nc.vector.scalar_tensor_tensor(
                    out=dst[:, 2 : 2 + N], in0=w, scalar=1.0, in1=lp,
                    op0=Alu.mult, op1=Alu.mult, accum_out=rsum_t[j][:, :],
                )
                # side pipeline (all off the DVE critical path except recip)
                nc.gpsimd.tensor_scalar(
                    out=rg_t[j], in0=rsum_t[j], scalar1=1.0, scalar2=1.0 / KC,
                    op0=Alu.max, op1=Alu.mult,
                )
                nc.vector.reciprocal(out=rinv_t[j], in_=rg_t[j])
                nc.gpsimd.tensor_scalar(
                    out=isd_t[j], in0=rsum_t[j], scalar1=0.0, scalar2=None,
                    op0=Alu.is_equal,
                )
                nc.scalar.activation(out=lnrg_t[j], in_=rg_t[j], func=Act.Ln)
                nc.gpsimd.tensor_add(out=acc[:, :], in0=acc[:, :], in1=lnrg_t[j][:, :])
            elif t % R == 0:
                # renorm correction using rsum(t-2)
                nc.vector.scalar_tensor_tensor(
                    out=dst[:, 2 : 2 + N], in0=w, scalar=rinv_t[j - 1][:, :], in1=lp,
                    op0=Alu.mult, op1=Alu.mult,
                )
            else:
                nc.vector.tensor_mul(out=dst[:, 2 : 2 + N], in0=w, in1=lp)

        def refresh(t):
            # overlap resync + scale alignment + dead-row adoption
            j = t // R - 1
            dst = a_pads[t % 2]
            psA = psum_r.tile([128, 1], f32, tag="psA", name=f"psA{(t//R)%2}")
            nc.tensor.matmul(psA, lhsT=shacc[:, :], rhs=acc[:, :], start=True, stop=True)
            delta = tmp.tile([128, 1], f32, tag="dl", name=f"dl{(t//R)%2}")
            nc.vector.tensor_sub(out=delta, in0=psA[:, :], in1=acc[:, :])
            # acc += delta * is_dead  (dead rows adopt upstream scale)
            nc.vector.scalar_tensor_tensor(
                out=acc[:, :], in0=delta, scalar=isd_t[j][:, :], in1=acc[:, :],
                op0=Alu.mult, op1=Alu.add,
            )
            # dm = delta*isd - delta; ratio = exp(-dm) = exp(delta*(1-isd))
            dm = tmp.tile([128, 1], f32, tag="dm", name=f"dm{(t//R)%2}")
            nc.vector.scalar_tensor_tensor(
                out=dm, in0=delta, scalar=isd_t[j][:, :], in1=delta,
                op0=Alu.mult, op1=Alu.subtract,
            )
            ratio = tmp.tile([128, 1], f32, tag="ra", name=f"ra{(t//R)%2}")
            nc.scalar.activation(out=ratio, in_=dm, func=Act.Exp, scale=-1.0)
            psS = psum_r.tile([128, W], f32, tag="psS", name=f"psS{(t//R)%2}")
            nc.tensor.matmul(
                psS, lhsT=shst[0:96, :], rhs=dst[0:96, 2 + N - W : 2 + N],
                start=True, stop=True,
            )
            nc.scalar.activation(
                out=dst[:, 2 : 2 + W], in_=psS[:, :], func=Act.Copy, scale=ratio[:, :]
            )

        # ---------- DP over time ----------
        for c in range(T // TC):
            pt = pstream.tile([128, TC, N], bf16, tag="pt", name=f"pt{c%4}")
            o, h = divmod(c, OCT // TC)
            for k in range(NCH):
                nc.sync.dma_start(
                    out=pt[32 * k : 32 * (k + 1), :, :],
                    in_=p_oct[o][:, h * TC : (h + 1) * TC, 33 * k : 33 * k + N],
                )
            if c == 0:
                # alpha_0: only s=0 (blank) and s=1 (first label) reachable
                nc.vector.tensor_copy(
                    out=a_pads[0][0:32, 2 + W : 4 + W], in_=pt[0:32, 0, W : W + 2]
                )
            for tl in range(1 if c == 0 else 0, TC):
                t = c * TC + tl
                step(t, pt[:, tl, :])
                if t % R == 0:
                    refresh(t)

        # ---------- epilogue: loss = -(ln(a127+a128) + acc - T*ln(KP)) -----
        a_fin = a_pads[(T - 1) % 2]
        likt = persist.tile([128, 1], f32, tag="likt")
        nc.vector.tensor_add(
            out=likt[96:128, :], in0=a_fin[96:128, 2 + W + 28 : 3 + W + 28],
            in1=a_fin[96:128, 3 + W + 28 : 4 + W + 28],
        )
        lnlik = persist.tile([128, 1], f32, tag="lnlik")
        nc.scalar.activation(out=lnlik[96:128, :], in_=likt[96:128, :], func=Act.Ln)
        tot = persist.tile([128, 1], f32, tag="tot")
        nc.vector.tensor_add(out=tot[96:128, :], in0=lnlik[96:128, :], in1=acc[96:128, :])
        out_t = persist.tile([128, 1], f32, tag="outt")
        nc.vector.tensor_scalar(
            out=out_t[96:128, :], in0=tot[96:128, :], scalar1=-1.0,
            scalar2=float(T) * float(np.log(np.float64(KP))),
            op0=Alu.mult, op1=Alu.add,
        )
        nc.sync.dma_start(out=loss[:, :], in_=out_t[96:128, :])

    nc.finalize()
    return nc


def _host_prep_core(y_true_c: np.ndarray):
    """Tiny index-preprocessing of y_true: one-hot gather matrix (scaled by
    KP) and the packed skip mask."""
    ext = np.full((BPC, S), BLANK, np.int32)
    ext[:, 1::2] = y_true_c
    g = np.zeros((BPC, C, SEXT), np.float32)
    g[:, :, W : W + S] = (
        ext[:, None, :] == np.arange(C, dtype=np.int32)[None, :, None]
    ) * np.float32(KP)
    g = np.ascontiguousarray(g.transpose(1, 0, 2).reshape(C, BPC * SEXT))
    m2f = np.zeros((BPC, S), np.float32)
    m2f[:, 3::2] = (y_true_c[:, 1:] != y_true_c[:, :-1]).astype(np.float32)
    m2r = np.zeros((128, N), np.float32)
    for k in range(NCH):
        for j in range(N):
            s = 33 * k - W + j
            if 0 <= s < S:
                m2r[32 * k : 32 * (k + 1), j] = m2f[:, s]
    return g, m2r


def _np_bf16():
    import ml_dtypes

    return ml_dtypes.bfloat16


_NC = None
LAST_RESULT = None


def kernel(y_true: np.ndarray, y_pred: np.ndarray) -> np.ndarray:
    global _NC, LAST_RESULT
    if _NC is None:
        _NC = _build()
    bfdt = _np_bf16()
    y_true = np.asarray(y_true, dtype=np.int32)
    y_pred = np.ascontiguousarray(np.asarray(y_pred, dtype=np.float32))
    ident = np.eye(128, dtype=np.float32)
    shst = np.zeros((128, 128), np.float32)
    for cc in range(96):
        shst[cc, cc + 32] = 1.0
    shacc = shst.copy()
    for cc in range(32):
        shacc[cc, cc] = 1.0
    in_maps = []
    for i in range(NCORES):
        sl = slice(i * BPC, (i + 1) * BPC)
        g, m2r = _host_prep_core(y_true[sl])
        in_maps.append(
            {
                "y_pred": y_pred[sl],
                "g_all": np.ascontiguousarray(g.astype(bfdt)),
                "m2mask": np.ascontiguousarray(m2r.astype(bfdt)),
                "ident": ident,
                "shst": np.ascontiguousarray(shst.astype(bfdt)),
                "shacc": shacc,
            }
        )
    res = run_bass_kernel_spmd(_NC, in_maps, core_ids=list(range(NCORES)))
    LAST_RESULT = res
    return np.concatenate([r["loss"] for r in res.results], axis=0)


# revision 8
# speedup vs baseline: 1.6708x; 1.0888x over previous
"""CTC loss kernel for Trainium2 (Bass/Tile), 8-core data parallel.

Linear-space CTC forward DP with periodic per-row renormalization:

    a_t[s] = (a[s] + a[s-1] + m2[s]*a[s-2]) * ptil_t[s]

where ptil = (y_pred + EPS) * KP, KP ~ e^{E[-dloss/dt]} chosen so the row
magnitude is drift-free on average.  Every R=8 steps each partition row is
rescaled by KC/max(rowsum, 1) (rowsum from the stt accumulator two steps
earlier, reciprocal on DVE), and the applied log-scales accumulate into a
per-row f32 accumulator; the final loss is -(ln(a[127]+a[128]) + acc -
T*ln(KP)).  All hot-loop tensors are bf16 (2x DVE mode); states that fall
~90 nats below their row maximum flush to zero, which the 2e-2 rel-err
budget tolerates by a wide margin (validated: max rel err ~2e-4).

Layout: 129 states packed as 4 chunks x 32 batches across 128 partitions.
Each row holds [2 zero pads | 16 overlap | 33 real] = 51 bf16 cols, so the
s-1/s-2 shifts stay in-lane.  The overlap is recomputed redundantly and
drifts 2 cols/step from the pads; every 8 steps a PE shift-matmul copies the
upstream chunk's top-16 states into the downstream overlap, rescaled by
exp(acc_upstream - acc_this) via an ACT per-partition-scaled copy.  Rows
whose states are still all-zero (unreachable chunks) instead adopt the
upstream accumulator so arriving values always land in f32 range.

The per-symbol probs ptil[b,t,s] are gathered on-device by TensorEngine
matmuls against one-hot matrices G[c,(b,sx)] = KP*(c==ext[b,sx]) in bf16,
staged through a DRAM scratch, and streamed back in the packed layout.
"""

import numpy as np

import concourse.bass as bass
import concourse.tile as tile
from concourse import bacc
from concourse import mybir
from concourse.bass_utils import run_bass_kernel_spmd
from contextlib import ExitStack

B, T, C, L = 256, 1024, 128, 64
NCORES = 8
BPC = B // NCORES          # 32 batch rows per core
S = 2 * L + 1              # 129 extended states
NCH = 4                    # state chunks per batch
W = 32                     # overlap states per chunk
N = W + 33                 # 49 computed states per row
SEXT = W + S + 3           # 148: padded per-batch state axis in the gather
R = 16                     # renorm + refresh period
BLANK = C - 1
EPS = 1e-7
KP = 108.0                 # folded into G: ptil = (y+EPS)*KP, E[step drift]~0
KC = float(2.0 ** 30)      # renorm target row sum
TC = 64                    # DP stream chunk: time steps per SBUF tile
OCT = 128                  # pregather granularity: time steps per matmul

f32 = mybir.dt.float32
bf16 = mybir.dt.bfloat16
Alu = mybir.AluOpType
Act = mybir.ActivationFunctionType

# Only Copy / Exp / Ln are used, all present in the single
# "natural_log_exp_and_others" table.  Blank every other table so the
# act-table placement pass settles on the combined table once (avoids a
# 1.3us table load on every Exp<->Ln transition).
_orig_get_act_tables = bacc.get_activation_tables


def _patched_get_act_tables(arch):
    tabs = _orig_get_act_tables(arch)
    keep = "natural_log_exp_and_others"
    if keep in tabs:
        tabs = {n: (fs if n == keep else set()) for n, fs in tabs.items()}
    return tabs


bacc.get_activation_tables = _patched_get_act_tables


def _build() -> bass.Bass:
    nc = bacc.Bacc()
    y_pred = nc.dram_tensor("y_pred", [BPC, T, C], f32, kind="ExternalInput")
    g_in = nc.dram_tensor("g_all", [C, BPC * SEXT], bf16, kind="ExternalInput")
    m2_in = nc.dram_tensor("m2mask", [128, N], bf16, kind="ExternalInput")
    id_in = nc.dram_tensor("ident", [128, 128], f32, kind="ExternalInput")
    shst_in = nc.dram_tensor("shst", [128, 128], bf16, kind="ExternalInput")
    shacc_in = nc.dram_tensor("shacc", [128, 128], f32, kind="ExternalInput")
    loss = nc.dram_tensor("loss", [BPC, 1], f32, kind="ExternalOutput")

    with tile.TileContext(nc) as tc, ExitStack() as ctx:
        persist = ctx.enter_context(tc.tile_pool(name="persist", bufs=1))
        tmp = ctx.enter_context(tc.tile_pool(name="tmp", bufs=3))
        ysb = ctx.enter_context(tc.tile_pool(name="ysb", bufs=3))
        ytp = ctx.enter_context(tc.tile_pool(name="ytp", bufs=3))
        pstream = ctx.enter_context(tc.tile_pool(name="pstream", bufs=3))
        psum_tp = ctx.enter_context(tc.tile_pool(name="psum_tp", bufs=2, space="PSUM"))
        psum_pp = ctx.enter_context(tc.tile_pool(name="psum_pp", bufs=2, space="PSUM"))
        psum_r = ctx.enter_context(tc.tile_pool(name="psum_r", bufs=2, space="PSUM"))
        dram = ctx.enter_context(tc.tile_pool(name="dram", bufs=1, space="DRAM"))

        # ---------- static inputs ----------
        ident = persist.tile([128, 128], f32, tag="ident")
        nc.sync.dma_start(out=ident, in_=id_in[:, :])
        g_all = persist.tile([C, BPC * SEXT], bf16, tag="gall")
        nc.sync.dma_start(out=g_all, in_=g_in[:, :])
        m2 = persist.tile([128, N], bf16, tag="m2")
        nc.sync.dma_start(out=m2, in_=m2_in[:, :])
        shst = persist.tile([128, 128], bf16, tag="shst")
        nc.sync.dma_start(out=shst, in_=shst_in[:, :])
        shacc = persist.tile([128, 128], f32, tag="shacc")
        nc.sync.dma_start(out=shacc, in_=shacc_in[:, :])


        # ---------- pregather: ptil[b,t,sx] = (y_pred[b,t,ext[b,sx]]+EPS)*KP
        p_oct = [
            dram.tile([BPC, OCT, SEXT], bf16, tag=f"oct{o}", name=f"p_oct{o}")
            for o in range(T // OCT)
        ]
        for o in range(T // OCT):
            for b in range(BPC):
                y_sb = ysb.tile([OCT, C], f32, tag="y")
                nc.sync.dma_start(out=y_sb, in_=y_pred[b, o * OCT : (o + 1) * OCT, :])
                yT_ps = psum_tp.tile([C, OCT], f32, tag="tp")
                nc.tensor.transpose(yT_ps, y_sb, ident)
                yT_sb = ytp.tile([C, OCT], bf16, tag="yT")
                nc.scalar.activation(out=yT_sb, in_=yT_ps, func=Act.Copy)
                p_ps = psum_pp.tile([OCT, SEXT], f32, tag="pp")
                nc.tensor.matmul(
                    p_ps, lhsT=yT_sb, rhs=g_all[:, b * SEXT : (b + 1) * SEXT],
                    start=True, stop=True,
                )
                p_sb = ytp.tile([OCT, SEXT], bf16, tag="psb")
                nc.scalar.activation(
                    out=p_sb, in_=p_ps, func=Act.Copy, bias=float(KP * EPS)
                )
                nc.scalar.dma_start(out=p_oct[o][b, :, :], in_=p_sb)

        # ---------- DP state ----------
        a_pads = [
            persist.tile([128, N + 2], bf16, tag=f"alpha{i}", name=f"alpha{i}")
            for i in range(2)
        ]
        nc.vector.memset(a_pads[0], 0.0)
        nc.vector.memset(a_pads[1], 0.0)
        acc = persist.tile([128, 1], f32, tag="acc")
        nc.vector.memset(acc, 0.0)
        nrs = 2 * (T // (2 * R)) + 2
        rsum_t = [persist.tile([128, 1], f32, tag=f"rs{i%4}", name=f"rs{i}") for i in range(nrs)]
        rg_t = [persist.tile([128, 1], f32, tag=f"rg{i%4}", name=f"rg{i}") for i in range(nrs)]
        rinv_t = [persist.tile([128, 1], f32, tag=f"ri{i%4}", name=f"ri{i}") for i in range(nrs)]
        isd_t = [persist.tile([128, 1], f32, tag=f"is{i%4}", name=f"is{i}") for i in range(nrs)]
        lnrg_t = [persist.tile([128, 1], f32, tag=f"ln{i%4}", name=f"ln{i}") for i in range(nrs)]

        def step(t, lp):
            src = a_pads[(t + 1) % 2]
            dst = a_pads[t % 2]
            a0 = src[:, 2 : 2 + N]
            a1 = src[:, 1 : 1 + N]
            a2 = src[:, 0:N]
            u = tmp.tile([128, N], bf16, tag="u", name=f"u{t%4}")
            nc.vector.tensor_add(out=u, in0=a0, in1=a1)
            t2 = tmp.tile([128, N], bf16, tag="t2", name=f"t2_{t%4}")
            nc.vector.tensor_mul(out=t2, in0=a2, in1=m2)
            w = tmp.tile([128, N], bf16, tag="w", name=f"w{t%4}")
            nc.vector.tensor_add(out=w, in0=u, in1=t2)
            j = t // R
            if t % R == R - 4 and t + 4 < T:
                # emit row sum for the correction four steps later
                nc.vector.scalar_tensor_tensor(
                    out=dst[:, 2 : 2 + N], in0=w, scalar=1.0, in1=lp,
                    op0=Alu.mult, op1=Alu.mult, accum_out=rsum_t[j][:, :],
                )
                # side pipeline (all off the DVE critical path except recip)
                nc.gpsimd.tensor_scalar(
                    out=rg_t[j], in0=rsum_t[j], scalar1=1.0, scalar2=1.0 / KC,
                    op0=Alu.max, op1=Alu.mult,
                )
                nc.vector.reciprocal(out=rinv_t[j], in_=rg_t[j])
                nc.gpsimd.tensor_scalar(
                    out=isd_t[j], in0=rsum_t[j], scalar1=0.0, scalar2=None,
                    op0=Alu.is_equal,
                )
                nc.scalar.activation(out=lnrg_t[j], in_=rg_t[j], func=Act.Ln)
                nc.gpsimd.tensor_add(out=acc[:, :], in0=acc[:, :], in1=lnrg_t[j][:, :])
            elif t % R == 0:
                # renorm correction using rsum(t-4)
                nc.vector.scalar_tensor_tensor(
                    out=dst[:, 2 : 2 + N], in0=w, scalar=rinv_t[j - 1][:, :], in1=lp,
                    op0=Alu.mult, op1=Alu.mult,
                )
            else:
                nc.vector.tensor_mul(out=dst[:, 2 : 2 + N], in0=w, in1=lp)

        def refresh(t):
            # overlap resync + scale alignment + dead-row adoption
            j = t // R - 1
            dst = a_pads[t % 2]
            psA = psum_r.tile([128, 1], f32, tag="psA", name=f"psA{(t//R)%2}")
            nc.tensor.matmul(psA, lhsT=shacc[:, :], rhs=acc[:, :], start=True, stop=True)
            delta = tmp.tile([128, 1], f32, tag="dl", name=f"dl{(t//R)%2}")
            nc.vector.tensor_sub(out=delta, in0=psA[:, :], in1=acc[:, :])
            # acc += delta * is_dead  (dead rows adopt upstream scale)
            nc.vector.scalar_tensor_tensor(
                out=acc[:, :], in0=delta, scalar=isd_t[j][:, :], in1=acc[:, :],
                op0=Alu.mult, op1=Alu.add,
            )
            # dm = delta*isd - delta; ratio = exp(-dm) = exp(delta*(1-isd))
            dm = tmp.tile([128, 1], f32, tag="dm", name=f"dm{(t//R)%2}")
            nc.vector.scalar_tensor_tensor(
                out=dm, in0=delta, scalar=isd_t[j][:, :], in1=delta,
                op0=Alu.mult, op1=Alu.subtract,
            )
            ratio = tmp.tile([128, 1], f32, tag="ra", name=f"ra{(t//R)%2}")
            nc.scalar.activation(out=ratio, in_=dm, func=Act.Exp, scale=-1.0)
            psS = psum_r.tile([128, W], f32, tag="psS", name=f"psS{(t//R)%2}")
            nc.tensor.matmul(
                psS, lhsT=shst[0:96, :], rhs=dst[0:96, 2 + N - W : 2 + N],
                start=True, stop=True,
            )
            nc.scalar.activation(
                out=dst[:, 2 : 2 + W], in_=psS[:, :], func=Act.Copy, scale=ratio[:, :]
            )

        # ---------- DP over time ----------
        for c in range(T // TC):
            pt = pstream.tile([128, TC, N], bf16, tag="pt", name=f"pt{c%4}")
            o, h = divmod(c, OCT // TC)
            for k in range(NCH):
                nc.sync.dma_start(
                    out=pt[32 * k : 32 * (k + 1), :, :],
                    in_=p_oct[o][:, h * TC : (h + 1) * TC, 33 * k : 33 * k + N],
                )
            if c == 0:
                # alpha_0: only s=0 (blank) and s=1 (first label) reachable
                nc.vector.tensor_copy(
                    out=a_pads[0][0:32, 2 + W : 4 + W], in_=pt[0:32, 0, W : W + 2]
                )
            for tl in range(1 if c == 0 else 0, TC):
                t = c * TC + tl
                step(t, pt[:, tl, :])
                if t % R == 0:
                    refresh(t)

        # ---------- epilogue: loss = -(ln(a127+a128) + acc - T*ln(KP)) -----
        a_fin = a_pads[(T - 1) % 2]
        likt = persist.tile([128, 1], f32, tag="likt")
        nc.vector.tensor_add(
            out=likt[96:128, :], in0=a_fin[96:128, 2 + W + 28 : 3 + W + 28],
            in1=a_fin[96:128, 3 + W + 28 : 4 + W + 28],
        )
        lnlik = persist.tile([128, 1], f32, tag="lnlik")
        nc.scalar.activation(out=lnlik[96:128, :], in_=likt[96:128, :], func=Act.Ln)
        tot = persist.tile([128, 1], f32, tag="tot")
        nc.vector.tensor_add(out=tot[96:128, :], in0=lnlik[96:128, :], in1=acc[96:128, :])
        out_t = persist.tile([128, 1], f32, tag="outt")
        nc.vector.tensor_scalar(
            out=out_t[96:128, :], in0=tot[96:128, :], scalar1=-1.0,
            scalar2=float(T) * float(np.log(np.float64(KP))),
            op0=Alu.mult, op1=Alu.add,
        )
        nc.sync.dma_start(out=loss[:, :], in_=out_t[96:128, :])

    nc.finalize()
    return nc


def _host_prep_core(y_true_c: np.ndarray):
    """Tiny index-preprocessing of y_true: one-hot gather matrix (scaled by
    KP) and the packed skip mask."""
    ext = np.full((BPC, S), BLANK, np.int32)
    ext[:, 1::2] = y_true_c
    g = np.zeros((BPC, C, SEXT), np.float32)
    g[:, :, W : W + S] = (
        ext[:, None, :] == np.arange(C, dtype=np.int32)[None, :, None]
    ) * np.float32(KP)
    g = np.ascontiguousarray(g.transpose(1, 0, 2).reshape(C, BPC * SEXT))
    m2f = np.zeros((BPC, S), np.float32)
    m2f[:, 3::2] = (y_true_c[:, 1:] != y_true_c[:, :-1]).astype(np.float32)
    m2r = np.zeros((128, N), np.float32)
    for k in range(NCH):
        for j in range(N):
            s = 33 * k - W + j
            if 0 <= s < S:
                m2r[32 * k : 32 * (k + 1), j] = m2f[:, s]
    return g, m2r


def _np_bf16():
    import ml_dtypes

    return ml_dtypes.bfloat16


_NC = None
LAST_RESULT = None


def kernel(y_true: np.ndarray, y_pred: np.ndarray) -> np.ndarray:
    global _NC, LAST_RESULT
    if _NC is None:
        _NC = _build()
    bfdt = _np_bf16()
    y_true = np.asarray(y_true, dtype=np.int32)
    y_pred = np.ascontiguousarray(np.asarray(y_pred, dtype=np.float32))
    ident = np.eye(128, dtype=np.float32)
    shst = np.zeros((128, 128), np.float32)
    for cc in range(96):
        shst[cc, cc + 32] = 1.0
    shacc = shst.copy()
    for cc in range(32):
        shacc[cc, cc] = 1.0
    in_maps = []
    for i in range(NCORES):
        sl = slice(i * BPC, (i + 1) * BPC)
        g, m2r = _host_prep_core(y_true[sl])
        in_maps.append(
            {
                "y_pred": y_pred[sl],
                "g_all": np.ascontiguousarray(g.astype(bfdt)),
                "m2mask": np.ascontiguousarray(m2r.astype(bfdt)),
                "ident": ident,
                "shst": np.ascontiguousarray(shst.astype(bfdt)),
                "shacc": shacc,
            }
        )
    res = run_bass_kernel_spmd(_NC, in_maps, core_ids=list(range(NCORES)))
    LAST_RESULT = res
    return np.concatenate([r["loss"] for r in res.results], axis=0)


# revision 11
# speedup vs baseline: 1.8557x; 1.1106x over previous
"""CTC loss kernel for Trainium2 (Bass/Tile), 8-core data parallel.

Linear-space CTC forward DP with periodic per-row renormalization:

    a_t[s] = (a[s] + a[s-1] + m2[s]*a[s-2]) * ptil_t[s]

where ptil = (y_pred + EPS) * KP, KP ~ e^{E[-dloss/dt]} chosen so the row
magnitude is drift-free on average.  Every R=8 steps each partition row is
rescaled by KC/max(rowsum, 1) (rowsum from the stt accumulator two steps
earlier, reciprocal on DVE), and the applied log-scales accumulate into a
per-row f32 accumulator; the final loss is -(ln(a[127]+a[128]) + acc -
T*ln(KP)).  All hot-loop tensors are bf16 (2x DVE mode); states that fall
~90 nats below their row maximum flush to zero, which the 2e-2 rel-err
budget tolerates by a wide margin (validated: max rel err ~2e-4).

Layout: 129 states packed as 4 chunks x 32 batches across 128 partitions.
Each row holds [2 zero pads | 16 overlap | 33 real] = 51 bf16 cols, so the
s-1/s-2 shifts stay in-lane.  The overlap is recomputed redundantly and
drifts 2 cols/step from the pads; every 8 steps a PE shift-matmul copies the
upstream chunk's top-16 states into the downstream overlap, rescaled by
exp(acc_upstream - acc_this) via an ACT per-partition-scaled copy.  Rows
whose states are still all-zero (unreachable chunks) instead adopt the
upstream accumulator so arriving values always land in f32 range.

The per-symbol probs ptil[b,t,s] are gathered on-device by TensorEngine
matmuls against one-hot matrices G[c,(b,sx)] = KP*(c==ext[b,sx]) in bf16,
staged through a DRAM scratch, and streamed back in the packed layout.
"""

import numpy as np

import concourse.bass as bass
import concourse.tile as tile
from concourse import bacc
from concourse import mybir
from concourse.bass_utils import run_bass_kernel_spmd
from contextlib import ExitStack

B, T, C, L = 256, 1024, 128, 64
NCORES = 8
BPC = B // NCORES          # 32 batch rows per core
S = 2 * L + 1              # 129 extended states
NCH = 4                    # state chunks per batch
W = 32                     # overlap states per chunk
N = W + 33                 # 49 computed states per row
SEXT = W + S + 3           # 148: padded per-batch state axis in the gather
R = 16                     # renorm + refresh period
BLANK = C - 1
EPS = 1e-7
KP = 108.0                 # folded into G: ptil = (y+EPS)*KP, E[step drift]~0
KC = float(2.0 ** 30)      # renorm target row sum
TC = 64                    # DP stream chunk: time steps per SBUF tile
OCT = 128                  # pregather granularity: time steps per matmul

f32 = mybir.dt.float32
bf16 = mybir.dt.bfloat16
Alu = mybir.AluOpType
Act = mybir.ActivationFunctionType

# Only Copy / Exp / Ln are used, all present in the single
# "natural_log_exp_and_others" table.  Blank every other table so the
# act-table placement pass settles on the combined table once (avoids a
# 1.3us table load on every Exp<->Ln transition).
_orig_get_act_tables = bacc.get_activation_tables


def _patched_get_act_tables(arch):
    tabs = _orig_get_act_tables(arch)
    keep = "natural_log_exp_and_others"
    if keep in tabs:
        tabs = {n: (fs if n == keep else set()) for n, fs in tabs.items()}
    return tabs


bacc.get_activation_tables = _patched_get_act_tables


def _build() -> bass.Bass:
    nc = bacc.Bacc()
    y_pred = nc.dram_tensor("y_pred", [BPC, T, C], f32, kind="ExternalInput")
    g_in = nc.dram_tensor("g_all", [C, BPC * SEXT], bf16, kind="ExternalInput")
    m2_in = nc.dram_tensor("m2mask", [128, N], bf16, kind="ExternalInput")
    id_in = nc.dram_tensor("ident", [128, 128], f32, kind="ExternalInput")
    shst_in = nc.dram_tensor("shst", [128, 128], bf16, kind="ExternalInput")
    shacc_in = nc.dram_tensor("shacc", [128, 128], f32, kind="ExternalInput")
    loss = nc.dram_tensor("loss", [BPC, 1], f32, kind="ExternalOutput")

    with tile.TileContext(nc) as tc, ExitStack() as ctx:
        persist = ctx.enter_context(tc.tile_pool(name="persist", bufs=1))
        tmp = ctx.enter_context(tc.tile_pool(name="tmp", bufs=3))
        ysb = ctx.enter_context(tc.tile_pool(name="ysb", bufs=6))
        ytp = ctx.enter_context(tc.tile_pool(name="ytp", bufs=6))
        pstream = ctx.enter_context(tc.tile_pool(name="pstream", bufs=3))
        psum_tp = ctx.enter_context(tc.tile_pool(name="psum_tp", bufs=3, space="PSUM"))
        psum_pp = ctx.enter_context(tc.tile_pool(name="psum_pp", bufs=3, space="PSUM"))
        psum_r = ctx.enter_context(tc.tile_pool(name="psum_r", bufs=1, space="PSUM"))
        dram = ctx.enter_context(tc.tile_pool(name="dram", bufs=1, space="DRAM"))

        # ---------- static inputs ----------
        ident = persist.tile([128, 128], f32, tag="ident")
        nc.sync.dma_start(out=ident, in_=id_in[:, :])
        g_all = persist.tile([C, BPC * SEXT], bf16, tag="gall")
        nc.sync.dma_start(out=g_all, in_=g_in[:, :])
        m2 = persist.tile([128, N], bf16, tag="m2")
        nc.sync.dma_start(out=m2, in_=m2_in[:, :])
        shst = persist.tile([128, 128], bf16, tag="shst")
        nc.sync.dma_start(out=shst, in_=shst_in[:, :])
        shacc = persist.tile([128, 128], f32, tag="shacc")
        nc.sync.dma_start(out=shacc, in_=shacc_in[:, :])


        # ---------- pregather: ptil[b,t,sx] = (y_pred[b,t,ext[b,sx]]+EPS)*KP
        p_oct = [
            dram.tile([BPC, OCT, SEXT], bf16, tag=f"oct{o}", name=f"p_oct{o}")
            for o in range(T // OCT)
        ]
        for o in range(T // OCT):
            for b in range(BPC):
                y_sb = ysb.tile([OCT, C], f32, tag="y")
                nc.sync.dma_start(out=y_sb, in_=y_pred[b, o * OCT : (o + 1) * OCT, :])
                yT_ps = psum_tp.tile([C, OCT], f32, tag="tp")
                nc.tensor.transpose(yT_ps, y_sb, ident)
                yT_sb = ytp.tile([C, OCT], bf16, tag="yT")
                nc.scalar.activation(out=yT_sb, in_=yT_ps, func=Act.Copy)
                p_ps = psum_pp.tile([OCT, SEXT], f32, tag="pp")
                nc.tensor.matmul(
                    p_ps, lhsT=yT_sb, rhs=g_all[:, b * SEXT : (b + 1) * SEXT],
                    start=True, stop=True,
                )
                p_sb = ytp.tile([OCT, SEXT], bf16, tag="psb")
                nc.scalar.activation(
                    out=p_sb, in_=p_ps, func=Act.Copy, bias=float(KP * EPS)
                )
                nc.gpsimd.dma_start(out=p_oct[o][b, :, :], in_=p_sb)

        # ---------- DP state ----------
        a_pads = [
            persist.tile([128, N + 2], bf16, tag=f"alpha{i}", name=f"alpha{i}")
            for i in range(2)
        ]
        nc.vector.memset(a_pads[0], 0.0)
        nc.vector.memset(a_pads[1], 0.0)
        acc = persist.tile([128, 1], f32, tag="acc")
        nc.vector.memset(acc, 0.0)
        nrs = 2 * (T // (2 * R)) + 2
        rsum_t = [persist.tile([128, 1], f32, tag=f"rs{i%4}", name=f"rs{i}") for i in range(nrs)]
        rg_t = [persist.tile([128, 1], f32, tag=f"rg{i%4}", name=f"rg{i}") for i in range(nrs)]
        rinv_t = [persist.tile([128, 1], f32, tag=f"ri{i%4}", name=f"ri{i}") for i in range(nrs)]
        isd_t = [persist.tile([128, 1], f32, tag=f"is{i%4}", name=f"is{i}") for i in range(nrs)]
        lnrg_t = [persist.tile([128, 1], f32, tag=f"ln{i%4}", name=f"ln{i}") for i in range(nrs)]

        def step(t, lp):
            src = a_pads[(t + 1) % 2]
            dst = a_pads[t % 2]
            a0 = src[:, 2 : 2 + N]
            a1 = src[:, 1 : 1 + N]
            a2 = src[:, 0:N]
            u = tmp.tile([128, N], bf16, tag="u", name=f"u{t%4}")
            nc.vector.tensor_add(out=u, in0=a0, in1=a1)
            t2 = tmp.tile([128, N], bf16, tag="t2", name=f"t2_{t%4}")
            nc.vector.tensor_mul(out=t2, in0=a2, in1=m2)
            w = tmp.tile([128, N], bf16, tag="w", name=f"w{t%4}")
            nc.vector.tensor_add(out=w, in0=u, in1=t2)
            j = t // R
            if t % R == R - 4 and t + 4 < T:
                # emit row sum for the correction four steps later
                nc.vector.scalar_tensor_tensor(
                    out=dst[:, 2 : 2 + N], in0=w, scalar=1.0, in1=lp,
                    op0=Alu.mult, op1=Alu.mult, accum_out=rsum_t[j][:, :],
                )
                # side pipeline (all off the DVE critical path except recip)
                nc.gpsimd.tensor_scalar(
                    out=rg_t[j], in0=rsum_t[j], scalar1=1.0, scalar2=1.0 / KC,
                    op0=Alu.max, op1=Alu.mult,
                )
                nc.vector.reciprocal(out=rinv_t[j], in_=rg_t[j])
                nc.gpsimd.tensor_scalar(
                    out=isd_t[j], in0=rsum_t[j], scalar1=0.0, scalar2=None,
                    op0=Alu.is_equal,
                )
                nc.scalar.activation(out=lnrg_t[j], in_=rg_t[j], func=Act.Ln)
                nc.gpsimd.tensor_add(out=acc[:, :], in0=acc[:, :], in1=lnrg_t[j][:, :])
            elif t % R == 0:
                # renorm correction using rsum(t-4)
                nc.vector.scalar_tensor_tensor(
                    out=dst[:, 2 : 2 + N], in0=w, scalar=rinv_t[j - 1][:, :], in1=lp,
                    op0=Alu.mult, op1=Alu.mult,
                )
            else:
                nc.vector.tensor_mul(out=dst[:, 2 : 2 + N], in0=w, in1=lp)

        def refresh(t):
            # overlap resync + scale alignment + dead-row adoption
            j = t // R - 1
            dst = a_pads[t % 2]
            psA = psum_r.tile([128, 1], f32, tag="psA", name=f"psA{(t//R)%2}")
            nc.tensor.matmul(psA, lhsT=shacc[:, :], rhs=acc[:, :], start=True, stop=True)
            delta = tmp.tile([128, 1], f32, tag="dl", name=f"dl{(t//R)%2}")
            nc.vector.tensor_sub(out=delta, in0=psA[:, :], in1=acc[:, :])
            # acc += delta * is_dead  (dead rows adopt upstream scale)
            nc.vector.scalar_tensor_tensor(
                out=acc[:, :], in0=delta, scalar=isd_t[j][:, :], in1=acc[:, :],
                op0=Alu.mult, op1=Alu.add,
            )
            # dm = delta*isd - delta; ratio = exp(-dm) = exp(delta*(1-isd))
            dm = tmp.tile([128, 1], f32, tag="dm", name=f"dm{(t//R)%2}")
            nc.vector.scalar_tensor_tensor(
                out=dm, in0=delta, scalar=isd_t[j][:, :], in1=delta,
                op0=Alu.mult, op1=Alu.subtract,
            )
            ratio = tmp.tile([128, 1], f32, tag="ra", name=f"ra{(t//R)%2}")
            nc.scalar.activation(out=ratio, in_=dm, func=Act.Exp, scale=-1.0)
            psS = psum_r.tile([128, W], f32, tag="psS", name=f"psS{(t//R)%2}")
            nc.tensor.matmul(
                psS, lhsT=shst[0:96, :], rhs=dst[0:96, 2 + N - W : 2 + N],
                start=True, stop=True,
            )
            nc.scalar.activation(
                out=dst[:, 2 : 2 + W], in_=psS[:, :], func=Act.Copy, scale=ratio[:, :]
            )

        # ---------- DP over time ----------
        for c in range(T // TC):
            pt = pstream.tile([128, TC, N], bf16, tag="pt", name=f"pt{c%4}")
            o, h = divmod(c, OCT // TC)
            for k in range(NCH):
                nc.sync.dma_start(
                    out=pt[32 * k : 32 * (k + 1), :, :],
                    in_=p_oct[o][:, h * TC : (h + 1) * TC, 33 * k : 33 * k + N],
                )
            if c == 0:
                # alpha_0: only s=0 (blank) and s=1 (first label) reachable
                nc.vector.tensor_copy(
                    out=a_pads[0][0:32, 2 + W : 4 + W], in_=pt[0:32, 0, W : W + 2]
                )
            for tl in range(1 if c == 0 else 0, TC):
                t = c * TC + tl
                step(t, pt[:, tl, :])
                if t % R == 0:
                    refresh(t)

        # ---------- epilogue: loss = -(ln(a127+a128) + acc - T*ln(KP)) -----
        a_fin = a_pads[(T - 1) % 2]
        likt = persist.tile([128, 1], f32, tag="likt")
        nc.vector.tensor_add(
            out=likt[96:128, :], in0=a_fin[96:128, 2 + W + 28 : 3 + W + 28],
            in1=a_fin[96:128, 3 + W + 28 : 4 + W + 28],
        )
        lnlik = persist.tile([128, 1], f32, tag="lnlik")
        nc.scalar.activation(out=lnlik[96:128, :], in_=likt[96:128, :], func=Act.Ln)
        tot = persist.tile([128, 1], f32, tag="tot")
        nc.vector.tensor_add(out=tot[96:128, :], in0=lnlik[96:128, :], in1=acc[96:128, :])
        out_t = persist.tile([128, 1], f32, tag="outt")
        nc.vector.tensor_scalar(
            out=out_t[96:128, :], in0=tot[96:128, :], scalar1=-1.0,
            scalar2=float(T) * float(np.log(np.float64(KP))),
            op0=Alu.mult, op1=Alu.add,
        )
        nc.sync.dma_start(out=loss[:, :], in_=out_t[96:128, :])

    nc.finalize()
    return nc


def _host_prep_core(y_true_c: np.ndarray):
    """Tiny index-preprocessing of y_true: one-hot gather matrix (scaled by
    KP) and the packed skip mask."""
    ext = np.full((BPC, S), BLANK, np.int32)
    ext[:, 1::2] = y_true_c
    g = np.zeros((BPC, C, SEXT), np.float32)
    g[:, :, W : W + S] = (
        ext[:, None, :] == np.arange(C, dtype=np.int32)[None, :, None]
    ) * np.float32(KP)
    g = np.ascontiguousarray(g.transpose(1, 0, 2).reshape(C, BPC * SEXT))
    m2f = np.zeros((BPC, S), np.float32)
    m2f[:, 3::2] = (y_true_c[:, 1:] != y_true_c[:, :-1]).astype(np.float32)
    m2r = np.zeros((128, N), np.float32)
    for k in range(NCH):
        for j in range(N):
            s = 33 * k - W + j
            if 0 <= s < S:
                m2r[32 * k : 32 * (k + 1), j] = m2f[:, s]
    return g, m2r


def _np_bf16():
    import ml_dtypes

    return ml_dtypes.bfloat16


_NC = None
LAST_RESULT = None


def kernel(y_true: np.ndarray, y_pred: np.ndarray) -> np.ndarray:
    global _NC, LAST_RESULT
    if _NC is None:
        _NC = _build()
    bfdt = _np_bf16()
    y_true = np.asarray(y_true, dtype=np.int32)
    y_pred = np.ascontiguousarray(np.asarray(y_pred, dtype=np.float32))
    ident = np.eye(128, dtype=np.float32)
    shst = np.zeros((128, 128), np.float32)
    for cc in range(96):
        shst[cc, cc + 32] = 1.0
    shacc = shst.copy()
    for cc in range(32):
        shacc[cc, cc] = 1.0
    in_maps = []
    for i in range(NCORES):
        sl = slice(i * BPC, (i + 1) * BPC)
        g, m2r = _host_prep_core(y_true[sl])
        in_maps.append(
            {
                "y_pred": y_pred[sl],
                "g_all": np.ascontiguousarray(g.astype(bfdt)),
                "m2mask": np.ascontiguousarray(m2r.astype(bfdt)),
                "ident": ident,
                "shst": np.ascontiguousarray(shst.astype(bfdt)),
                "shacc": shacc,
            }
        )
    res = run_bass_kernel_spmd(_NC, in_maps, core_ids=list(range(NCORES)))
    LAST_RESULT = res
    return np.concatenate([r["loss"] for r in res.results], axis=0)
